# revision 19
# baseline (speedup 1.0000x reference)
"""Trainium2 Bass kernel for nn_CrossAttention (single-head NxN attention + proj + InstanceNorm + residual).

Sharding: 8 cores = (batch b in 0..3) x (query-half h in 0..1).
Each core computes its half of the query tokens for one batch; the
InstanceNorm statistics (over the full 4096 tokens) are combined across
the core pair by exchanging raw bn_stats records with a tiny AllGather
and aggregating all 8 records locally.

Precision: matmul operands in fp16 (safe: every tensor here has tiny
dynamic range - |scores|<~6, p=exp(s)<~250, |q|,|k|,|v|<~6), all
accumulation (PSUM), softmax denominators, InstanceNorm statistics and
the residual path in fp32.  x1 is loaded as fp16 (|x1|~N(0,1), rounding
~5e-4 abs) and upcast on-chip for the residual.

Self-contained: hardcodes shapes B=4, C=256, D=H=W=16 (N=4096), Cr=32.
"""

import numpy as np

import concourse.bass as bass
import concourse.mybir as mybir
import concourse.tile as tile
from concourse import bacc
from concourse.bass_utils import run_bass_kernel_spmd
from concourse.masks import make_identity

B, C, N, Cr = 4, 256, 4096, 32
NH = N // 2  # query tokens per core
EPS = 1e-5
SCALE = float(Cr) ** -0.5
FP32 = mybir.dt.float32
FP16 = mybir.dt.float16

N_CORES = 8
REPLICA_GROUPS = [[0, 1], [2, 3], [4, 5], [6, 7]]

IT = 512                   # i-tile width (query columns processed together)
N_ITILES = NH // IT        # 4
JBLK = 128                 # j-block (rows per QK matmul output)
N_JBLK = N // JBLK         # 32
JB_PER_BURST = 2           # j-blocks per burst; each row-tiled QK matmul owns a full PSUM bank
N_JBURSTS = N_JBLK // JB_PER_BURST  # 16

AF = mybir.ActivationFunctionType
ALU = mybir.AluOpType

LAST_RESULTS = None  # BassKernelResults of the most recent run (for test harness)


def build_nc(use_collective=True):
    nc = bacc.Bacc("TRN2", num_devices=N_CORES, name="xattn",
                   target_bir_lowering=False)

    x1h_d = nc.dram_tensor("x1h", [C, NH], FP16, kind="ExternalInput").ap()
    x2b_d = nc.dram_tensor("x2b", [C, N], FP16, kind="ExternalInput").ap()
    wqT_d = nc.dram_tensor("wqT", [C, Cr], FP16, kind="ExternalInput").ap()
    wkT_d = nc.dram_tensor("wkT", [C, Cr], FP16, kind="ExternalInput").ap()
    wvT_d = nc.dram_tensor("wvT", [C, C], FP16, kind="ExternalInput").ap()
    wpT_d = nc.dram_tensor("wpT", [C, C], FP16, kind="ExternalInput").ap()
    out_d = nc.dram_tensor("out", [C, NH], FP16, kind="ExternalOutput").ap()

    with tile.TileContext(nc) as tc:
        build_body(tc, x1h_d, x2b_d, wqT_d, wkT_d, wvT_d, wpT_d, out_d,
                   use_collective)
    nc.compile()
    return nc


def build_body(tc, x1h_d, x2b_d, wqT_d, wkT_d, wvT_d, wpT_d, out_d,
               use_collective=True):
    nc = tc.nc
    from contextlib import ExitStack

    with ExitStack() as ctx:
        persist = ctx.enter_context(tc.tile_pool(name="persist", bufs=1))
        ptp = ctx.enter_context(tc.tile_pool(name="ptp", bufs=3))
        sm = ctx.enter_context(tc.tile_pool(name="sm", bufs=4))
        sm2 = ctx.enter_context(tc.tile_pool(name="sm2", bufs=2))
        qkp = ctx.enter_context(tc.tile_pool(name="qkp", bufs=2, space="PSUM"))
        avp = ctx.enter_context(tc.tile_pool(name="avp", bufs=4, space="PSUM"))
        dramp = ctx.enter_context(tc.tile_pool(name="dramp", bufs=1, space="DRAM"))

        # ---- constants -------------------------------------------------
        eps_sb = persist.tile([128, 1], FP32, tag="eps", name="eps_sb")
        nc.vector.memset(eps_sb, EPS)
        ident = persist.tile([128, 128], FP32, tag="ident", name="ident")
        make_identity(nc, ident)

        # ---- input loads, split across the two HWDGE queues ------------
        wk_sb, wv_sb, wq_sb, wp_sb = [], [], [], []
        for cc in range(2):
            w2 = persist.tile([128, Cr], FP16, tag=f"wk{cc}", name=f"wk_sb{cc}")
            nc.sync.dma_start(w2, wkT_d[128 * cc:128 * (cc + 1), :])
            wk_sb.append(w2)
            w3 = persist.tile([128, C], FP16, tag=f"wv{cc}", name=f"wv_sb{cc}")
            nc.scalar.dma_start(w3, wvT_d[128 * cc:128 * (cc + 1), :])
            wv_sb.append(w3)
        for cc in range(2):
            w1 = persist.tile([128, Cr], FP16, tag=f"wq{cc}", name=f"wq_sb{cc}")
            nc.scalar.dma_start(w1, wqT_d[128 * cc:128 * (cc + 1), :])
            wq_sb.append(w1)
        x2_sb = [persist.tile([128, N], FP16, tag=f"x2_{cc}", name=f"x2_sb{cc}")
                 for cc in range(2)]
        x1_hf = [persist.tile([128, NH], FP16, tag=f"x1f_{cc}",
                              name=f"x1_hf{cc}") for cc in range(2)]
        for ch in range(2):          # x2 cols [0:1024], [1024:2048] on sync
            sl = slice(1024 * ch, 1024 * (ch + 1))
            for cc in range(2):
                nc.sync.dma_start(x2_sb[cc][:, sl],
                                  x2b_d[128 * cc:128 * (cc + 1), sl])
        for ch in range(2):          # all of x1 (q + residual) on sync
            sl = slice(1024 * ch, 1024 * (ch + 1))
            for cc in range(2):
                nc.sync.dma_start(x1_hf[cc][:, sl],
                                  x1h_d[128 * cc:128 * (cc + 1), sl])
        for ch in range(2, 4):       # x2 cols [2048:3072], [3072:4096] on scalar
            sl = slice(1024 * ch, 1024 * (ch + 1))
            for cc in range(2):
                nc.scalar.dma_start(x2_sb[cc][:, sl],
                                    x2b_d[128 * cc:128 * (cc + 1), sl])
        for cc in range(2):
            w4 = persist.tile([128, C], FP16, tag=f"wp{cc}", name=f"wp_sb{cc}")
            nc.scalar.dma_start(w4, wpT_d[128 * cc:128 * (cc + 1), :])
            wp_sb.append(w4)

        # ---- warm-up collective: absorbs the rendezvous barrier and the
        # ncfw dispatch cold cost so the real stats AllGather is fast.
        # Gated (via a GpSimd copy) on the last x1 chunk so the barrier's
        # SDMA traffic doesn't throttle the input loads.
        if use_collective:
            warm_sb = persist.tile([128, 1], FP32, tag="warm", name="warm_sb")
            nc.gpsimd.tensor_copy(warm_sb, x1_hf[1][:, NH - 1:NH])
            warm_in = dramp.tile([128, 1], FP32, tag="warm_in", name="warm_in")
            warm_out = dramp.tile([2, 128, 1], FP32, tag="warm_out",
                                  name="warm_out")
            nc.sync.dma_start(warm_in, warm_sb)
            nc.gpsimd.collective_compute(
                "AllGather", ALU.bypass, replica_groups=REPLICA_GROUPS,
                ins=[warm_in.opt()], outs=[warm_out.opt()])

        # ---- prologue compute, ordered by chunk arrival ----------------
        vt = persist.tile([128, N_JBLK, C + 1], FP16, tag="vt", name="vt")
        nc.vector.memset(vt[:, :, C:C + 1], 1.0)
        k_rep = persist.tile([128, N], FP16, tag="krep", name="k_rep")
        q_rep = persist.tile([128, NH], FP16, tag="qrep", name="q_rep")

        def emit_k(jt):
            kp = avp.tile([128, 512], FP32, tag="av", name=f"kp{jt}")
            for ct in range(4):
                for cc in range(2):
                    nc.tensor.matmul(
                        kp[32 * ct:32 * (ct + 1), :], lhsT=wk_sb[cc],
                        rhs=x2_sb[cc][:, 512 * jt:512 * (jt + 1)],
                        start=(cc == 0), stop=(cc == 1),
                        tile_position=(0, 32 * ct))
            nc.vector.tensor_copy(k_rep[:, 512 * jt:512 * (jt + 1)], kp)

        def emit_v(jblk):
            vp = avp.tile([128, 512], FP32, tag="av", name=f"vp{jblk}")[:, 0:C]
            for cc in range(2):
                nc.tensor.matmul(
                    vp, lhsT=x2_sb[cc][:, 128 * jblk:128 * (jblk + 1)],
                    rhs=wv_sb[cc], start=(cc == 0), stop=(cc == 1))
            nc.vector.tensor_copy(vt[:, jblk, 0:C], vp)

        def emit_q(qt):
            qp = avp.tile([128, 512], FP32, tag="av", name=f"qp{qt}")
            for ct in range(4):
                for cc in range(2):
                    nc.tensor.matmul(
                        qp[32 * ct:32 * (ct + 1), :], lhsT=wq_sb[cc],
                        rhs=x1_hf[cc][:, 512 * qt:512 * (qt + 1)],
                        start=(cc == 0), stop=(cc == 1),
                        tile_position=(0, 32 * ct))
            nc.vector.tensor_copy(q_rep[:, 512 * qt:512 * (qt + 1)], qp)

        for ch in range(4):
            for jt in range(2 * ch, 2 * (ch + 1)):
                emit_k(jt)
            for jblk in range(8 * ch, 8 * (ch + 1)):
                emit_v(jblk)
            if ch == 0:
                emit_q(0)
                emit_q(1)
            if ch == 1:
                emit_q(2)
                emit_q(3)

        # ---- persistent attention outputs ------------------------------
        proj_sb = [persist.tile([128, NH], FP32, tag=f"proj{ob}", name=f"proj_sb{ob}")
                   for ob in range(2)]
        stats2 = persist.tile([128, 2, N_ITILES, 6], FP32, tag="stats2",
                              name="stats2")

        def emit_qk(it, jb):
            isl = slice(IT * it, IT * (it + 1))
            qk = qkp.tile([128, IT * JB_PER_BURST], FP32, tag="qk",
                          name=f"qk{it}_{jb}")
            for t in range(JB_PER_BURST):
                jblk = jb * JB_PER_BURST + t
                rt = t + 2 * (jb % 2)   # alternate row-groups between bursts
                nc.tensor.matmul(
                    qk[:, IT * t:IT * (t + 1)],
                    lhsT=k_rep[32 * rt:32 * (rt + 1),
                               JBLK * jblk:JBLK * (jblk + 1)],
                    rhs=q_rep[32 * rt:32 * (rt + 1), isl],
                    start=True, stop=True, tile_position=(32 * rt, 0))
            pt = ptp.tile([128, IT * JB_PER_BURST], FP16, tag="pt",
                          name=f"pt{it}_{jb}")
            nc.scalar.activation(out=pt, in_=qk, func=AF.Exp)
            return pt

        def emit_av(av_t, jb, pt):
            for t in range(JB_PER_BURST):
                jblk = jb * JB_PER_BURST + t
                for ib in range(4):
                    nc.tensor.matmul(
                        av_t[ib],
                        lhsT=pt[:, IT * t + 128 * ib:IT * t + 128 * (ib + 1)],
                        rhs=vt[:, jblk, :],
                        start=(jb == 0 and t == 0),
                        stop=(jb == N_JBURSTS - 1 and t == JB_PER_BURST - 1))

        def emit_epilogue(it, av_t):
            isl = slice(IT * it, IT * (it + 1))
            avc = [sm2.tile([128, IT], FP16, tag=f"avc{cc}", name=f"avc{it}_{cc}")
                   for cc in range(2)]
            for ib in range(4):
                rden = sm.tile([128, 1], FP32, tag="rden", name=f"rden{it}_{ib}")
                nc.vector.reciprocal(rden, av_t[ib][:, C:C + 1])
                avn = sm.tile([128, C], FP32, tag="avn", name=f"avn{it}_{ib}")
                nc.vector.tensor_scalar_mul(avn, in0=av_t[ib][:, 0:C], scalar1=rden)
                tp = avp.tile([128, 512], FP32, tag="av",
                              name=f"tp{it}_{ib}")[:, 0:C]
                nc.tensor.transpose(tp[:, 0:128], avn[:, 0:128], ident)
                nc.tensor.transpose(tp[:, 128:256], avn[:, 128:256], ident)
                for cc in range(2):
                    nc.vector.tensor_copy(avc[cc][:, 128 * ib:128 * (ib + 1)],
                                          tp[:, 128 * cc:128 * (cc + 1)])
            for ob in range(2):
                pj = avp.tile([128, IT], FP32, tag="av", name=f"pj{it}_{ob}")
                for cc in range(2):
                    nc.tensor.matmul(
                        pj, lhsT=wp_sb[cc][:, 128 * ob:128 * (ob + 1)],
                        rhs=avc[cc], start=(cc == 0), stop=(cc == 1))
                nc.vector.bn_stats(stats2[:, ob, it, :], pj)
                nc.vector.tensor_copy(proj_sb[ob][:, isl], pj)

        # ---- main attention loop, software-pipelined across bursts -----
        pt_hold = emit_qk(0, 0)
        for it in range(N_ITILES):
            av_t = [avp.tile([128, 512], FP32, tag="av",
                             name=f"av{it}_{ib}")[:, 0:C + 1]
                    for ib in range(4)]
            for jb in range(N_JBURSTS):
                last = (it == N_ITILES - 1 and jb == N_JBURSTS - 1)
                if not last:
                    nit, njb = (it, jb + 1) if jb + 1 < N_JBURSTS else (it + 1, 0)
                    pt_next = emit_qk(nit, njb)
                else:
                    pt_next = None
                emit_av(av_t, jb, pt_hold)
                pt_hold = pt_next
            emit_epilogue(it, av_t)

        # ---- pre-warm the Sqrt activation table while stats fly --------
        sq_warm = sm.tile([128, 1], FP32, tag="sqw", name="sq_warm")
        nc.scalar.activation(out=sq_warm, in_=eps_sb, func=AF.Sqrt)

        # ---- cross-core InstanceNorm stats: exchange raw bn_stats ------
        NREC = 2 * N_ITILES * 6
        cc8 = persist.tile([128, 2, 2, N_ITILES, 6], FP32, tag="cc8",
                           name="cc8")  # [p, ob, rank, itile, stat]
        if use_collective:
            ccin_dr = dramp.tile([128, NREC], FP32, tag="ccin_d",
                                 name="ccin_dr")
            ccout_dr = dramp.tile([2, 128, NREC], FP32,
                                  tag="ccout_d", name="ccout_dr")
            nc.sync.dma_start(ccin_dr,
                              stats2.rearrange("p o i s -> p (o i s)"))
            nc.gpsimd.collective_compute(
                "AllGather", ALU.bypass, replica_groups=REPLICA_GROUPS,
                ins=[ccin_dr.opt()], outs=[ccout_dr.opt()])
            # one strided DMA: [r, p, (o i s)] -> [p, o, r, i, s]
            nc.sync.dma_start(
                cc8, ccout_dr.rearrange("r p (o i s) -> p o r i s", o=2,
                                        i=N_ITILES))
        else:
            nc.vector.tensor_copy(cc8[:, :, 0], stats2)
            nc.vector.tensor_copy(cc8[:, :, 1], stats2)

        # ---- upcast x1 to fp32 during the collective wait --------------
        x1_f32 = []
        for cc in range(2):
            t32 = persist.tile([128, NH], FP32, tag=f"x132_{cc}",
                               name=f"x1_f32{cc}")
            for ch in range(2):
                sl = slice(1024 * ch, 1024 * (ch + 1))
                nc.vector.tensor_copy(t32[:, sl], x1_hf[cc][:, sl])
            x1_f32.append(t32)

        # ---- aggregate all 8 records per output half -------------------
        mvab = persist.tile([128, 2, 2], FP32, tag="mvab", name="mvab")
        for ob in range(2):
            nc.vector.bn_aggr(out=mvab[:, ob],
                              in_=cc8[:, ob].rearrange("p r i s -> p (r i) s"))
        # rstd = sqrt(1 / (var + eps)); negmr = -mean * rstd
        veps = sm.tile([128, 2], FP32, tag="veps", name="veps")
        nc.vector.tensor_scalar(out=veps, in0=mvab[:, :, 1], scalar1=EPS,
                                scalar2=None, op0=ALU.add)
        rvep = sm.tile([128, 2], FP32, tag="rvep", name="rvep")
        nc.vector.reciprocal(rvep, veps)
        rstd2 = persist.tile([128, 2], FP32, tag="rstd2", name="rstd2")
        nc.scalar.activation(out=rstd2, in_=rvep, func=AF.Sqrt)
        negmr = persist.tile([128, 2], FP32, tag="negmr", name="negmr")
        nc.vector.tensor_mul(negmr, mvab[:, :, 0], rstd2)
        nc.vector.tensor_scalar_mul(negmr, in0=negmr, scalar1=-1.0)

        # ---- normalize + residual + store ------------------------------
        for ch in range(4):
            sl = slice(512 * ch, 512 * (ch + 1))
            for ob in range(2):
                mean = negmr[:, ob:ob + 1]
                rstd = rstd2[:, ob:ob + 1]
                nt = sm.tile([128, 512], FP32, tag="nt", name=f"nt{ob}_{ch}")
                nc.scalar.activation(out=nt, in_=proj_sb[ob][:, sl],
                                     func=AF.Identity, bias=mean, scale=rstd)
                ot = sm.tile([128, 512], FP16, tag="ot", name=f"ot{ob}_{ch}")
                nc.vector.tensor_add(ot, nt, x1_f32[ob][:, sl])
                nc.sync.dma_start(out_d[128 * ob:128 * (ob + 1), sl], ot)


_nc_cache = None


def _get_nc():
    global _nc_cache
    if _nc_cache is None:
        _nc_cache = build_nc()
    return _nc_cache


def make_in_maps(x1, x2, wq, wk, wv, wp):
    x1f = np.asarray(x1, np.float32).reshape(B, C, N).astype(np.float16)
    x2f = np.asarray(x2, np.float32).reshape(B, C, N).astype(np.float16)
    wqT = np.ascontiguousarray(((np.asarray(wq, np.float32) * SCALE).T).astype(np.float16))
    wkT = np.ascontiguousarray((np.asarray(wk, np.float32).T).astype(np.float16))
    wvT = np.ascontiguousarray((np.asarray(wv, np.float32).T).astype(np.float16))
    wpT = np.ascontiguousarray((np.asarray(wp, np.float32).T).astype(np.float16))
    in_maps = []
    for core in range(N_CORES):
        b, h = core // 2, core % 2
        in_maps.append({
            "x1h": np.ascontiguousarray(x1f[b, :, h * NH:(h + 1) * NH]),
            "x2b": np.ascontiguousarray(x2f[b]),
            "wqT": wqT, "wkT": wkT, "wvT": wvT, "wpT": wpT,
        })
    return in_maps


def assemble_out(results):
    out = np.empty((B, C, N), np.float32)
    for core in range(N_CORES):
        b, h = core // 2, core % 2
        out[b, :, h * NH:(h + 1) * NH] = np.asarray(results[core]["out"], np.float32)
    return out.reshape(B, C, 16, 16, 16)


def kernel(**inputs):
    global LAST_RESULTS
    in_maps = make_in_maps(inputs["x1"], inputs["x2"], inputs["wq"],
                           inputs["wk"], inputs["wv"], inputs["wp"])
    res = run_bass_kernel_spmd(_get_nc(), in_maps, core_ids=list(range(N_CORES)))
    LAST_RESULTS = res
    return assemble_out(res.results)
